# revision 19
# baseline (speedup 1.0000x reference)
"""Trainium2 Bass kernel for nn_AttentionModeEncoder (B=4, S=2048, HID=1024, 16 heads x 64).

Sharding: 8 cores = 4 batches x 2 head-groups (8 heads / 512 features per core).

Design notes (R1 rev):
  - All transposes on HOST: xT, compacted xkT, WqT/WkT/WvT, WoT are DRAM
    inputs (bf16).  Mask compaction on host: only unmasked keys (~1044 of
    2048) shipped, padded to KP=1152; pad slots get -1e9 bias.
  - Everything bf16 on the PE; PSUM accumulation fp32; softmax denominator
    fp32 via the [V | ones] augmented AV stationary matrix.
  - Phase B is a 3-engine software pipeline: scores for iteration i+1 are
    emitted one full iteration ahead so the ACT engine's exp stream never
    stalls; AV matmuls trail each exp per key-tile; DVE does the
    reciprocal-normalize per iteration.
  - DMA triggers are split across the two HWDGE queues (sync + scalar):
    sync carries the A1-critical xk/wv/wk and half the y writes; scalar
    carries the consts pack, wq, wo, gated xq slices, and the other half
    of y.  y is written bf16 (host sums the two partials per batch).
Per core (batch b, head-group g):
  A1: K^T/V projections from xkT, V in [k, head, d|ones] layout.
  A2: Q^T projection from xT (jt-outer), bias on DVE.
  B:  pipelined scores/exp/AV/normalize per (head, 1024-wide q chunk).
  C:  partial out-projection y^T = WoT^T @ attn^T, bias, bf16 DMA out.
"""


import os
import sys
import numpy as np
from contextlib import ExitStack

for _p in ("/opt/trn_rl_repo", "/root/.axon_site/_ro/trn_rl_repo"):
    if os.path.isdir(_p) and _p not in sys.path:
        sys.path.insert(0, _p)

import ml_dtypes
import concourse.bass as bass
import concourse.bacc as bacc
import concourse.mybir as mybir
import concourse.tile as tile
from concourse import library_config

B, S, HID = 4, 2048, 1024
JC = 512                 # features per core (8 heads)
KP = 1152                # compacted+padded key slots (9 k-tiles)
NKT = KP // 128          # 9
NCORES = 8
FP = mybir.dt.float32
FR = mybir.dt.float32r
BF = mybir.dt.bfloat16
F8 = mybir.dt.float8e4
DR = mybir.MatmulPerfMode.DoubleRow
MULT = mybir.AluOpType.mult
ADD = mybir.AluOpType.add
EXP = mybir.ActivationFunctionType.Exp
IDENT = mybir.ActivationFunctionType.Identity

# consts pack layout (fp32 columns)
CP_BVR = 0      # [128, 512] bv replicated
CP_MASK = 512   # [128, 9]   mask bias per key tile
CP_BQ = 521     # [128, 4]   bq transposed
CP_BK = 525     # [128, 4]   bk transposed
CP_BO = 529     # [128, 8]   bo transposed
CP_W = 537

TRACE = False
LAST_RESULTS = {}


def build_nc():
    nc = bacc.Bacc()
    xT = nc.declare_dram_parameter("xT", [HID, S], BF, isOutput=False)
    xkT = nc.declare_dram_parameter("xkT", [HID, KP], BF, isOutput=False)
    cpack = nc.declare_dram_parameter("cpack", [128, CP_W], FP, isOutput=False)
    wqT = nc.declare_dram_parameter("wqT", [HID, JC], BF, isOutput=False)
    wkT = nc.declare_dram_parameter("wkT", [HID, JC], BF, isOutput=False)
    wvT = nc.declare_dram_parameter("wvT", [HID, JC], BF, isOutput=False)
    woT = nc.declare_dram_parameter("woT", [JC, HID], BF, isOutput=False)
    y = nc.declare_dram_parameter("y", [HID, S], BF, isOutput=True)

    with tile.TileContext(nc) as tc, ExitStack() as ctx:
        const = ctx.enter_context(tc.tile_pool(name="const", bufs=1))
        mid = ctx.enter_context(tc.tile_pool(name="mid", bufs=1))
        wop = ctx.enter_context(tc.tile_pool(name="wop", bufs=1))
        # LIFO pool stacks: a1 (innermost) closes after A1, a12 after A2
        a12stack = ExitStack()
        xqp = a12stack.enter_context(tc.tile_pool(name="xqp", bufs=1))
        wqp = a12stack.enter_context(tc.tile_pool(name="wqp", bufs=1))
        a1stack = ExitStack()
        xkp = a1stack.enter_context(tc.tile_pool(name="xkp", bufs=1))
        wkvp = a1stack.enter_context(tc.tile_pool(name="wkvp", bufs=1))

        # persistent activations
        QTs = mid.tile([128, 4, S], BF)          # [j-in-tile, jt, t]   16KB/part
        KTs = mid.tile([128, 4, KP], BF)         # [j-in-tile, jt, kc]   9KB
        vaug = mid.tile([128, NKT, 8, 128], BF)  # [kc, kt, head, d|ones@64|pad]
        nc.gpsimd.memset(vaug[:, :, :, 64:65], 1.0)
        outT = mid.tile([128, 4, S], BF)         # attn out^T [c, ct, t] 16KB

        # --- A1-critical loads split across both HWDGE queues so the first
        # v_proj inputs (xk slice 0 + wv) transfer in parallel ---------------
        xk = xkp.tile([128, 8, KP], BF)          # 18KB, freed after A1
        xk_r = xkT.rearrange("(it p) k -> p it k", p=128)
        wv_sb = wkvp.tile([128, 8, JC], BF)
        nc.sync.dma_start(out=wv_sb[:], in_=wvT.rearrange("(it p) j -> p it j", p=128))
        nc.scalar.dma_start(out=xk[:, :, 0:256], in_=xk_r[:, :, 0:256])
        wk_sb = wkvp.tile([128, 8, JC], BF)
        for k0, k1 in ((256, 512), (512, 832), (832, KP)):
            nc.sync.dma_start(out=xk[:, :, k0:k1], in_=xk_r[:, :, k0:k1])
        nc.sync.dma_start(out=wk_sb[:], in_=wkT.rearrange("(it p) j -> p it j", p=128))

        # --- everything else on the scalar HWDGE queue ---------------------
        cp = const.tile([128, CP_W], FP)
        nc.scalar.dma_start(out=cp[:], in_=cpack[:, :])
        wq_sb = wqp.tile([128, 8, JC], BF)
        nc.scalar.dma_start(out=wq_sb[:], in_=wqT.rearrange("(it p) j -> p it j", p=128))
        wo_sb = wop.tile([128, 4, HID], BF)
        nc.scalar.dma_start(out=wo_sb[:], in_=woT.rearrange("(ct p) o -> p ct o", p=128))
        xq = xqp.tile([128, 8, S], BF)           # 32KB, freed after A2

        bvr = cp[:, CP_BVR:CP_BVR + 512]
        maskA = cp[:, CP_MASK:CP_MASK + NKT]
        bqt = cp[:, CP_BQ:CP_BQ + 4]
        bkt = cp[:, CP_BK:CP_BK + 4]
        bot = cp[:, CP_BO:CP_BO + 8]

        # ---------------- Phase A1: K^T and V projections (compacted keys) ---
        with ExitStack() as actx:
            psA = actx.enter_context(tc.tile_pool(name="psA", bufs=2, space="PSUM"))

            # V in natural [kc, head, d] layout: out[kc, j] = sum_i xkT[i,kc] WvT[i,j]
            def v_proj(kc):
                ps = psA.tile([128, 8, 64], FP, tag="psv")
                for it in range(8):
                    nc.tensor.matmul(
                        ps[:],
                        lhsT=xk[:, it, kc * 128:(kc + 1) * 128],
                        rhs=wv_sb[:, it, :],
                        start=(it == 0), stop=(it == 7),
                    )
                nc.vector.tensor_tensor(vaug[:, kc, :, 0:64], ps[:], bvr, ADD)

            # stagger the 4MB xq load across A1: each xq slice's DMA is gated
            # (WAW via gpsimd copy) on successive V tiles so startup bandwidth
            # goes to the A1-critical xk/wv/wk bytes first.
            xT_r = xT.rearrange("(it p) t -> p it t", p=128)

            def xq_gate(tq, kc):
                t0 = tq * 512
                nc.gpsimd.tensor_copy(
                    out=xq[0:1, 0, t0:t0 + 1], in_=vaug[0:1, kc, 0, 0:1]
                )
                nc.scalar.dma_start(
                    out=xq[:, :, t0:t0 + 512], in_=xT_r[:, :, t0:t0 + 512]
                )

            v_proj(0)
            xq_gate(0, 0)
            for kc in range(1, NKT):
                v_proj(kc)
                if kc in (2, 4, 6):
                    xq_gate(kc // 2, kc)

            # K^T[j, kc]: 3 chunks of 384 columns
            for jt in range(4):
                for cc in range(3):
                    c0 = cc * 384
                    ps = psA.tile([128, 384], FP, tag="psk")
                    for it in range(8):
                        nc.tensor.matmul(
                            ps[:],
                            lhsT=wk_sb[:, it, jt * 128:(jt + 1) * 128],
                            rhs=xk[:, it, c0:c0 + 384],
                            start=(it == 0), stop=(it == 7),
                        )
                    nc.vector.tensor_scalar_add(
                        KTs[:, jt, c0:c0 + 384], ps[:], bkt[:, jt:jt + 1]
                    )

        a1stack.close()

        # ---------------- Phase A2: Q^T projection for jt=0 only -------------
        # (jt=1..3 are computed inside phase B's PE slack, one chain per
        # iteration, through the avpool rotation)
        with ExitStack() as actx:
            psQ = actx.enter_context(tc.tile_pool(name="psQ", bufs=2, space="PSUM"))
            for tq in range(4):
                t0 = tq * 512
                ps = psQ.tile([128, 512], FP, tag="psq")
                for it in range(8):
                    nc.tensor.matmul(
                        ps[:],
                        lhsT=wq_sb[:, it, 0:128],
                        rhs=xq[:, it, t0:t0 + 512],
                        start=(it == 0), stop=(it == 7),
                    )
                nc.vector.tensor_scalar_add(
                    QTs[:, 0, t0:t0 + 512], ps[:], bqt[:, 0:1]
                )

        # gpsimd: switch from the standard ucode library to `attn` for the
        # partition_broadcast used in phase B (after the last std-lib copy)
        nc.gpsimd.load_library(library_config.attn)

        # ---------------- Phase B: attention (3-engine pipeline) -------------
        # One N=1024 matmul per key-tile for both scores and AV (2 PSUM
        # banks per write).  Per exp slot the PE does one half-array scores
        # matmul + one full-array AV matmul of the previous iteration, which
        # keeps instantaneous PE power smooth (the hw governor clamps the PE
        # to 50% util when bursty full-array chains push it over budget).
        with ExitStack() as bctx:
            ptp = bctx.enter_context(tc.tile_pool(name="ptp", bufs=2))
            rp = bctx.enter_context(tc.tile_pool(name="rp", bufs=2))
            spool = bctx.enter_context(tc.tile_pool(name="spool", bufs=2, space="PSUM"))
            avpool = bctx.enter_context(tc.tile_pool(name="avpool", bufs=2, space="PSUM"))

            iters = [(h, qc) for h in range(8) for qc in range(2)]
            sp_t, PT_t, avp_t = {}, {}, {}

            def scores(i, kt):
                h, qc = iters[i]
                jt, p0, q0 = h // 2, (h % 2) * 64, qc * 1024
                sp = spool.tile([128, 1024], FP, tag="sp", name="sp")
                sp_t[(i, kt)] = sp
                for qq in range(2):
                    nc.tensor.matmul(
                        sp[:, qq * 512:(qq + 1) * 512],
                        lhsT=KTs[p0:p0 + 64, jt, kt * 128:(kt + 1) * 128],
                        rhs=QTs[p0:p0 + 64, jt, q0 + qq * 512:q0 + (qq + 1) * 512],
                        start=True, stop=True,
                    )

            def expf(i, kt):
                if kt == 0:
                    PT_t[i] = ptp.tile([128, NKT, 1024], BF, tag="PT", name="PT")
                nc.scalar.activation(
                    PT_t[i][:, kt, :], sp_t.pop((i, kt))[:], EXP,
                    bias=maskA[:, kt:kt + 1], scale=0.125,
                )

            def av_chain(i, qq):
                h, _ = iters[i]
                if qq == 0:
                    avp_t[i] = avpool.tile([128, 1024], FP, tag="avp", name="avp")
                for kt in range(NKT):
                    nc.tensor.matmul(
                        avp_t[i][0:65, qq * 512:(qq + 1) * 512],
                        lhsT=vaug[:, kt, h, 0:65],
                        rhs=PT_t[i][:, kt, qq * 512:(qq + 1) * 512],
                        start=(kt == 0), stop=(kt == NKT - 1),
                        skip_group_check=True,
                    )

            def dencopy(i):
                # denominator row (avp row 64) PSUM -> SBUF (custom-DVE recip
                # must read SBUF on real hw)
                den1 = rp.tile([1, 1024], FP, tag="den1", name="den1")
                den_t[i] = den1
                nc.vector.tensor_copy(out=den1[:], in_=avp_t[i][64:65, :])

            def recip(i):
                rec1 = rp.tile([1, 1024], FP, tag="rec1", name="rec1")
                rec_t[i] = rec1
                nc.vector.reciprocal_approx_fast(rec1[:], den_t.pop(i)[:])

            def bcast(i):
                # broadcast rec1 [1,1024] to 64 partitions on the idle gpsimd
                rec1 = rec_t.pop(i)
                rb = rp.tile([64, 1024], FP, tag="rb", name="rb")
                rb_t[i] = rb
                nc.gpsimd.partition_broadcast(rb[:], rec1[:], channels=64)

            def normalize(i):
                h, qc = iters[i]
                jt, p0, q0 = h // 2, (h % 2) * 64, qc * 1024
                avp = avp_t.pop(i)
                nc.vector.tensor_tensor(
                    outT[p0:p0 + 64, jt, q0:q0 + 1024],
                    avp[0:64, :], rb_t.pop(i)[:], MULT,
                )
                PT_t.pop(i)

            qt_t = {}

            def q_chain(idx):
                # one deferred A2 chain (jt=1..3) in this iteration's PE slack
                jt, c = 1 + idx // 4, idx % 4
                tq0 = (c // 2) * 2
                if c % 2 == 0:
                    qt_t[jt, c // 2] = avpool.tile(
                        [128, 1024], FP, tag="avp", name="qt"
                    )
                qt = qt_t[jt, c // 2]
                t0 = (tq0 + c % 2) * 512
                for it in range(8):
                    nc.tensor.matmul(
                        qt[:, (c % 2) * 512:(c % 2) * 512 + 512],
                        lhsT=wq_sb[:, it, jt * 128:(jt + 1) * 128],
                        rhs=xq[:, it, t0:t0 + 512],
                        start=(it == 0), stop=(it == 7),
                        skip_group_check=True,
                    )
                if c % 2 == 1:
                    qt = qt_t.pop((jt, c // 2))
                    nc.vector.tensor_scalar_add(
                        QTs[:, jt, tq0 * 512:tq0 * 512 + 1024], qt[:],
                        bqt[:, jt:jt + 1],
                    )

            rec_t, rb_t, den_t = {}, {}, {}
            for i in range(17):
                for kt in range(NKT):
                    if i < 16:
                        scores(i, kt)
                        expf(i, kt)
                        if kt == 3 and i < 12:
                            q_chain(i)
                    if i >= 1:
                        if kt == 1:
                            av_chain(i - 1, 0)
                        elif kt == 5:
                            av_chain(i - 1, 1)
                        elif kt == 6:
                            dencopy(i - 1)
                            recip(i - 1)
                        elif kt == 7:
                            bcast(i - 1)
                        elif kt == 8:
                            normalize(i - 1)

        a12stack.close()

        # ---------------- Phase C: partial out-projection --------------------
        with ExitStack() as cctx:
            ypool = cctx.enter_context(tc.tile_pool(name="ypool", bufs=4))
            ypsum = cctx.enter_context(tc.tile_pool(name="ypsum", bufs=3, space="PSUM"))

            for ot in range(8):
                for tch in range(2):
                    t0 = tch * 1024
                    yps = ypsum.tile([128, 1024], FP, tag="yps")
                    for qq in range(2):
                        for ct in range(4):
                            nc.tensor.matmul(
                                yps[:, qq * 512:(qq + 1) * 512],
                                lhsT=wo_sb[:, ct, ot * 128:(ot + 1) * 128],
                                rhs=outT[:, ct, t0 + qq * 512:t0 + (qq + 1) * 512],
                                start=(ct == 0), stop=(ct == 3),
                            )
                    yt = ypool.tile([128, 1024], BF, tag="yt")
                    if (ot + tch) % 2 == 0:
                        nc.scalar.activation(
                            yt[:], yps[:], IDENT, bias=bot[:, ot:ot + 1], scale=1.0
                        )
                    else:
                        nc.vector.tensor_scalar_add(yt[:], yps[:], bot[:, ot:ot + 1])
                    eng = nc.sync if (ot + tch) % 2 == 0 else nc.scalar
                    eng.dma_start(
                        out=y[ot * 128:(ot + 1) * 128, t0:t0 + 1024], in_=yt[:],
                    )
    return nc


_NC = None


def _get_nc():
    global _NC
    if _NC is None:
        _NC = build_nc()
        _NC.finalize()   # run Bacc passes (reg alloc, wait splitting)
    return _NC


def make_in_maps(x, mask, Wq, bq, Wk, bk, Wv, bv, Wo, bo):
    x = np.asarray(x, np.float32)
    mask = np.asarray(mask)
    per_batch = []
    for b in range(B):
        xTb = np.ascontiguousarray(x[b].T)
        sel = np.flatnonzero(mask[b])[:KP]
        ku = len(sel)
        xkTb = np.zeros((HID, KP), np.float32)
        xkTb[:, :ku] = xTb[:, sel]
        mb = np.zeros(KP, np.float32)
        mb[ku:] = -1e9
        per_batch.append((xTb.astype(ml_dtypes.bfloat16),
                          xkTb.astype(ml_dtypes.bfloat16), mb))
    per_g = []
    for g in range(2):
        sl = slice(g * JC, (g + 1) * JC)
        cpack = np.zeros((128, CP_W), np.float32)
        cpack[:, CP_BVR:CP_BVR + 512] = np.asarray(bv)[sl].astype(np.float32)[None, :]
        cpack[:, CP_BQ:CP_BQ + 4] = np.asarray(bq)[sl].reshape(4, 128).T
        cpack[:, CP_BK:CP_BK + 4] = np.asarray(bk)[sl].reshape(4, 128).T
        bog = np.asarray(bo) if g == 0 else np.zeros(HID, np.float32)
        cpack[:, CP_BO:CP_BO + 8] = np.asarray(bog, np.float32).reshape(8, 128).T
        per_g.append({
            "wqT": np.ascontiguousarray(np.asarray(Wq)[sl].T.astype(ml_dtypes.bfloat16)),
            "wkT": np.ascontiguousarray(np.asarray(Wk)[sl].T.astype(ml_dtypes.bfloat16)),
            "wvT": np.ascontiguousarray(np.asarray(Wv)[sl].T.astype(ml_dtypes.bfloat16)),
            "woT": np.ascontiguousarray(
                np.asarray(Wo)[:, sl].T.astype(ml_dtypes.bfloat16)
            ),
            "cpack": cpack,
        })
    in_maps = []
    for c in range(NCORES):
        b, g = c // 2, c % 2
        xTb, xkTb, mb = per_batch[b]
        m = {"xT": xTb, "xkT": xkTb}
        m.update(per_g[g])
        cpk = per_g[g]["cpack"].copy()
        cpk[:, CP_MASK:CP_MASK + NKT] = mb.reshape(NKT, 128).T
        m["cpack"] = cpk
        in_maps.append(m)
    return in_maps


def kernel(x, mask, Wq, bq, Wk, bk, Wv, bv, Wo, bo):
    from concourse.bass_utils import run_bass_kernel_spmd

    nc = _get_nc()
    in_maps = make_in_maps(x, mask, Wq, bq, Wk, bk, Wv, bv, Wo, bo)
    kw = {}
    if TRACE:
        import shutil
        shutil.rmtree("/root/problem/trace_out", ignore_errors=True)
        os.makedirs("/root/problem/trace_out", exist_ok=True)
        kw = dict(tmpdir="/root/problem/trace_out")
    r = run_bass_kernel_spmd(nc, in_maps, list(range(NCORES)), trace=TRACE, **kw)
    LAST_RESULTS["exec_time_ns"] = r.exec_time_ns
    LAST_RESULTS["mean_exec_time_ns"] = r.mean_exec_time_ns
    y = np.empty((B, S, HID), np.float32)
    for b in range(B):
        y[b] = (np.asarray(r.results[2 * b]["y"], np.float32)
                + np.asarray(r.results[2 * b + 1]["y"], np.float32)).T
    return y


# revision 20
# speedup vs baseline: 1.1606x; 1.1606x over previous
"""Trainium2 Bass kernel for nn_AttentionModeEncoder (B=4, S=2048, HID=1024, 16 heads x 64).

Sharding: 8 cores = 4 batches x 2 head-groups (8 heads / 512 features per core).

Design notes (R1 rev):
  - All transposes on HOST: xT, compacted xkT, WqT/WkT/WvT, WoT are DRAM
    inputs (bf16).  Mask compaction on host: only unmasked keys (~1044 of
    2048) shipped, padded to KP=1152; pad slots get -1e9 bias.
  - Everything bf16 on the PE; PSUM accumulation fp32; softmax denominator
    fp32 via the [V | ones] augmented AV stationary matrix.
  - Phase B is a 3-engine software pipeline: scores for iteration i+1 are
    emitted one full iteration ahead so the ACT engine's exp stream never
    stalls; AV matmuls trail each exp per key-tile; DVE does the
    reciprocal-normalize per iteration.
  - DMA triggers are split across the two HWDGE queues (sync + scalar):
    sync carries the A1-critical xk/wv/wk and half the y writes; scalar
    carries the consts pack, wq, wo, gated xq slices, and the other half
    of y.  y is written bf16 (host sums the two partials per batch).
Per core (batch b, head-group g):
  A1: K^T/V projections from xkT, V in [k, head, d|ones] layout.
  A2: Q^T projection from xT (jt-outer), bias on DVE.
  B:  pipelined scores/exp/AV/normalize per (head, 1024-wide q chunk).
  C:  partial out-projection y^T = WoT^T @ attn^T, bias, bf16 DMA out.
"""


import os
import sys
import numpy as np
from contextlib import ExitStack

for _p in ("/opt/trn_rl_repo", "/root/.axon_site/_ro/trn_rl_repo"):
    if os.path.isdir(_p) and _p not in sys.path:
        sys.path.insert(0, _p)

import ml_dtypes
import concourse.bass as bass
import concourse.bacc as bacc
import concourse.mybir as mybir
import concourse.tile as tile
from concourse import library_config

B, S, HID = 4, 2048, 1024
JC = 512                 # features per core (8 heads)
KP = 1152                # compacted+padded key slots (9 k-tiles)
NKT = KP // 128          # 9
NCORES = 8
FP = mybir.dt.float32
FR = mybir.dt.float32r
BF = mybir.dt.bfloat16
F8 = mybir.dt.float8e4
DR = mybir.MatmulPerfMode.DoubleRow
MULT = mybir.AluOpType.mult
ADD = mybir.AluOpType.add
EXP = mybir.ActivationFunctionType.Exp
IDENT = mybir.ActivationFunctionType.Identity

# consts pack layout (fp32 columns)
CP_BVR = 0      # [128, 512] bv replicated
CP_MASK = 512   # [128, 9]   mask bias per key tile
CP_BQ = 521     # [128, 4]   bq transposed
CP_BK = 525     # [128, 4]   bk transposed
CP_BO = 529     # [128, 8]   bo transposed
CP_W = 537

TRACE = False
LAST_RESULTS = {}


def build_nc():
    nc = bacc.Bacc()
    xT = nc.declare_dram_parameter("xT", [HID, S], BF, isOutput=False)
    xkT = nc.declare_dram_parameter("xkT", [HID, KP], BF, isOutput=False)
    cpack = nc.declare_dram_parameter("cpack", [128, CP_W], FP, isOutput=False)
    wqT = nc.declare_dram_parameter("wqT", [HID, JC], BF, isOutput=False)
    wkT = nc.declare_dram_parameter("wkT", [HID, JC], BF, isOutput=False)
    wvT = nc.declare_dram_parameter("wvT", [HID, JC], BF, isOutput=False)
    woT = nc.declare_dram_parameter("woT", [JC, HID], BF, isOutput=False)
    y = nc.declare_dram_parameter("y", [HID, S], BF, isOutput=True)

    with tile.TileContext(nc) as tc, ExitStack() as ctx:
        const = ctx.enter_context(tc.tile_pool(name="const", bufs=1))
        mid = ctx.enter_context(tc.tile_pool(name="mid", bufs=1))
        wop = ctx.enter_context(tc.tile_pool(name="wop", bufs=1))
        # LIFO pool stacks: a1 (innermost) closes after A1, a12 after A2
        a12stack = ExitStack()
        xqp = a12stack.enter_context(tc.tile_pool(name="xqp", bufs=1))
        wqp = a12stack.enter_context(tc.tile_pool(name="wqp", bufs=1))
        a1stack = ExitStack()
        xkp = a1stack.enter_context(tc.tile_pool(name="xkp", bufs=1))
        wkvp = a1stack.enter_context(tc.tile_pool(name="wkvp", bufs=1))

        # persistent activations
        QTs = mid.tile([128, 4, S], BF)          # [j-in-tile, jt, t]   16KB/part
        KTs = mid.tile([128, 4, KP], BF)         # [j-in-tile, jt, kc]   9KB
        vaug = mid.tile([128, NKT, 8, 128], BF)  # [kc, kt, head, d|ones@64|pad]
        nc.gpsimd.memset(vaug[:, :, :, 64:65], 1.0)
        # head 7 (the last two B iterations) gets a full 64-wide ones block so
        # its finalize skips the gpsimd broadcast on the critical tail
        nc.gpsimd.memset(vaug[:, :, 7, 64:128], 1.0)
        outT = mid.tile([128, 4, S], BF)         # attn out^T [c, ct, t] 16KB

        # --- A1-critical loads split across both HWDGE queues so the first
        # v_proj inputs (xk slice 0 + wv) transfer in parallel ---------------
        xk = xkp.tile([128, 8, KP], BF)          # 18KB, freed after A1
        xk_r = xkT.rearrange("(it p) k -> p it k", p=128)
        wv_sb = wkvp.tile([128, 8, JC], BF)
        nc.sync.dma_start(out=wv_sb[:], in_=wvT.rearrange("(it p) j -> p it j", p=128))
        nc.scalar.dma_start(out=xk[:, :, 0:256], in_=xk_r[:, :, 0:256])
        wk_sb = wkvp.tile([128, 8, JC], BF)
        for k0, k1 in ((256, 512), (512, 832), (832, KP)):
            nc.sync.dma_start(out=xk[:, :, k0:k1], in_=xk_r[:, :, k0:k1])
        nc.sync.dma_start(out=wk_sb[:], in_=wkT.rearrange("(it p) j -> p it j", p=128))

        # --- everything else on the scalar HWDGE queue ---------------------
        cp = const.tile([128, CP_W], FP)
        nc.scalar.dma_start(out=cp[:], in_=cpack[:, :])
        wq_sb = wqp.tile([128, 8, JC], BF)
        nc.scalar.dma_start(out=wq_sb[:], in_=wqT.rearrange("(it p) j -> p it j", p=128))
        wo_sb = wop.tile([128, 4, HID], BF)
        nc.scalar.dma_start(out=wo_sb[:], in_=woT.rearrange("(ct p) o -> p ct o", p=128))
        xq = xqp.tile([128, 8, S], BF)           # 32KB, freed after A2

        bvr = cp[:, CP_BVR:CP_BVR + 512]
        maskA = cp[:, CP_MASK:CP_MASK + NKT]
        bqt = cp[:, CP_BQ:CP_BQ + 4]
        bkt = cp[:, CP_BK:CP_BK + 4]
        bot = cp[:, CP_BO:CP_BO + 8]

        # ---------------- Phase A1: K^T and V projections (compacted keys) ---
        with ExitStack() as actx:
            psA = actx.enter_context(tc.tile_pool(name="psA", bufs=2, space="PSUM"))

            # V in natural [kc, head, d] layout: out[kc, j] = sum_i xkT[i,kc] WvT[i,j]
            def v_proj(kc):
                ps = psA.tile([128, 8, 64], FP, tag="psv")
                for it in range(8):
                    nc.tensor.matmul(
                        ps[:],
                        lhsT=xk[:, it, kc * 128:(kc + 1) * 128],
                        rhs=wv_sb[:, it, :],
                        start=(it == 0), stop=(it == 7),
                    )
                nc.vector.tensor_tensor(vaug[:, kc, :, 0:64], ps[:], bvr, ADD)

            # stagger the 4MB xq load across A1: each xq slice's DMA is gated
            # (WAW via gpsimd copy) on successive V tiles so startup bandwidth
            # goes to the A1-critical xk/wv/wk bytes first.
            xT_r = xT.rearrange("(it p) t -> p it t", p=128)

            def xq_gate(tq, kc):
                t0 = tq * 512
                nc.gpsimd.tensor_copy(
                    out=xq[0:1, 0, t0:t0 + 1], in_=vaug[0:1, kc, 0, 0:1]
                )
                nc.scalar.dma_start(
                    out=xq[:, :, t0:t0 + 512], in_=xT_r[:, :, t0:t0 + 512]
                )

            v_proj(0)
            xq_gate(0, 0)
            for kc in range(1, NKT):
                v_proj(kc)
                if kc in (2, 4, 6):
                    xq_gate(kc // 2, kc)

            # K^T[j, kc]: 3 chunks of 384 columns
            for jt in range(4):
                for cc in range(3):
                    c0 = cc * 384
                    ps = psA.tile([128, 384], FP, tag="psk")
                    for it in range(8):
                        nc.tensor.matmul(
                            ps[:],
                            lhsT=wk_sb[:, it, jt * 128:(jt + 1) * 128],
                            rhs=xk[:, it, c0:c0 + 384],
                            start=(it == 0), stop=(it == 7),
                        )
                    nc.vector.tensor_scalar_add(
                        KTs[:, jt, c0:c0 + 384], ps[:], bkt[:, jt:jt + 1]
                    )

        a1stack.close()

        # ---------------- Phase A2: Q^T projection (jt-outer, bias on DVE) ---
        with ExitStack() as actx:
            psQ = actx.enter_context(tc.tile_pool(name="psQ", bufs=2, space="PSUM"))
            for jt in range(4):
                for tq in range(4):
                    t0 = tq * 512
                    ps = psQ.tile([128, 512], FP, tag="psq")
                    for it in range(8):
                        nc.tensor.matmul(
                            ps[:],
                            lhsT=wq_sb[:, it, jt * 128:(jt + 1) * 128],
                            rhs=xq[:, it, t0:t0 + 512],
                            start=(it == 0), stop=(it == 7),
                        )
                    nc.vector.tensor_scalar_add(
                        QTs[:, jt, t0:t0 + 512], ps[:], bqt[:, jt:jt + 1]
                    )

        a12stack.close()

        # gpsimd: switch from the standard ucode library to `attn` for the
        # partition_broadcast used in phase B (after the last std-lib copy)
        nc.gpsimd.load_library(library_config.attn)

        # ---------------- Phase B: attention (3-engine pipeline) -------------
        # One N=1024 matmul per key-tile for both scores and AV (2 PSUM
        # banks per write).  Per exp slot the PE does one half-array scores
        # matmul + one full-array AV matmul of the previous iteration, which
        # keeps instantaneous PE power smooth (the hw governor clamps the PE
        # to 50% util when bursty full-array chains push it over budget).
        with ExitStack() as bctx:
            ptp = bctx.enter_context(tc.tile_pool(name="ptp", bufs=2))
            rp = bctx.enter_context(tc.tile_pool(name="rp", bufs=2))
            spool = bctx.enter_context(tc.tile_pool(name="spool", bufs=2, space="PSUM"))
            avpool = bctx.enter_context(tc.tile_pool(name="avpool", bufs=2, space="PSUM"))

            iters = [(h, qc) for h in range(8) for qc in range(2)]
            sp_t, PT_t, avp_t = {}, {}, {}

            def scores(i, kt):
                h, qc = iters[i]
                jt, p0, q0 = h // 2, (h % 2) * 64, qc * 1024
                sp = spool.tile([128, 1024], FP, tag="sp", name="sp")
                sp_t[(i, kt)] = sp
                for qq in range(2):
                    nc.tensor.matmul(
                        sp[:, qq * 512:(qq + 1) * 512],
                        lhsT=KTs[p0:p0 + 64, jt, kt * 128:(kt + 1) * 128],
                        rhs=QTs[p0:p0 + 64, jt, q0 + qq * 512:q0 + (qq + 1) * 512],
                        start=True, stop=True,
                    )

            def expf(i, kt):
                if kt == 0:
                    PT_t[i] = ptp.tile([128, NKT, 1024], BF, tag="PT", name="PT")
                nc.scalar.activation(
                    PT_t[i][:, kt, :], sp_t.pop((i, kt))[:], EXP,
                    bias=maskA[:, kt:kt + 1], scale=0.125,
                )

            def av_chain(i, qq):
                h, _ = iters[i]
                wid = 128 if h == 7 else 65
                if qq == 0:
                    avp_t[i] = avpool.tile([128, 1024], FP, tag="avp", name="avp")
                for kt in range(NKT):
                    nc.tensor.matmul(
                        avp_t[i][0:wid, qq * 512:(qq + 1) * 512],
                        lhsT=vaug[:, kt, h, 0:wid],
                        rhs=PT_t[i][:, kt, qq * 512:(qq + 1) * 512],
                        start=(kt == 0), stop=(kt == NKT - 1),
                        skip_group_check=True,
                    )

            def dencopy(i):
                # denominator row(s) PSUM -> SBUF (custom-DVE recip must read
                # SBUF on real hw).  Head 7 has the full replicated block.
                h, _ = iters[i]
                rows = 64 if h == 7 else 1
                den1 = rp.tile([rows, 1024], FP, tag="den1", name="den1")
                den_t[i] = den1
                nc.vector.tensor_copy(out=den1[:], in_=avp_t[i][64:64 + rows, :])

            def recip(i):
                h, _ = iters[i]
                rows = 64 if h == 7 else 1
                rec1 = rp.tile([rows, 1024], FP, tag="rec1", name="rec1")
                rec_t[i] = rec1
                nc.vector.reciprocal_approx_fast(rec1[:], den_t.pop(i)[:])
                if h == 7:
                    rb_t[i] = rec_t.pop(i)

            def bcast(i):
                # broadcast rec1 [1,1024] to 64 partitions on the idle gpsimd
                if i in rb_t:
                    return        # head-7 fast path: already 64 rows
                rec1 = rec_t.pop(i)
                rb = rp.tile([64, 1024], FP, tag="rb", name="rb")
                rb_t[i] = rb
                nc.gpsimd.partition_broadcast(rb[:], rec1[:], channels=64)

            def normalize(i):
                h, qc = iters[i]
                jt, p0, q0 = h // 2, (h % 2) * 64, qc * 1024
                avp = avp_t.pop(i)
                nc.vector.tensor_tensor(
                    outT[p0:p0 + 64, jt, q0:q0 + 1024],
                    avp[0:64, :], rb_t.pop(i)[:], MULT,
                )
                PT_t.pop(i)

            rec_t, rb_t, den_t = {}, {}, {}
            for i in range(17):
                for kt in range(NKT):
                    if i < 16:
                        scores(i, kt)
                        expf(i, kt)
                    if i >= 1:
                        if kt == 1:
                            av_chain(i - 1, 0)
                        elif kt == 5:
                            av_chain(i - 1, 1)
                        elif kt == 6:
                            dencopy(i - 1)
                            recip(i - 1)
                        elif kt == 7:
                            bcast(i - 1)
                        elif kt == 8:
                            normalize(i - 1)

        # ---------------- Phase C: partial out-projection --------------------
        with ExitStack() as cctx:
            ypool = cctx.enter_context(tc.tile_pool(name="ypool", bufs=4))
            ypsum = cctx.enter_context(tc.tile_pool(name="ypsum", bufs=3, space="PSUM"))

            for ot in range(8):
                for tch in range(2):
                    t0 = tch * 1024
                    yps = ypsum.tile([128, 1024], FP, tag="yps")
                    for qq in range(2):
                        for ct in range(4):
                            nc.tensor.matmul(
                                yps[:, qq * 512:(qq + 1) * 512],
                                lhsT=wo_sb[:, ct, ot * 128:(ot + 1) * 128],
                                rhs=outT[:, ct, t0 + qq * 512:t0 + (qq + 1) * 512],
                                start=(ct == 0), stop=(ct == 3),
                            )
                    yt = ypool.tile([128, 1024], BF, tag="yt")
                    if (ot + tch) % 2 == 0:
                        nc.scalar.activation(
                            yt[:], yps[:], IDENT, bias=bot[:, ot:ot + 1], scale=1.0
                        )
                    else:
                        nc.vector.tensor_scalar_add(yt[:], yps[:], bot[:, ot:ot + 1])
                    eng = nc.sync if (ot + tch) % 2 == 0 else nc.scalar
                    eng.dma_start(
                        out=y[ot * 128:(ot + 1) * 128, t0:t0 + 1024], in_=yt[:],
                    )
    return nc


_NC = None


def _get_nc():
    global _NC
    if _NC is None:
        _NC = build_nc()
        _NC.finalize()   # run Bacc passes (reg alloc, wait splitting)
    return _NC


def make_in_maps(x, mask, Wq, bq, Wk, bk, Wv, bv, Wo, bo):
    x = np.asarray(x, np.float32)
    mask = np.asarray(mask)
    per_batch = []
    for b in range(B):
        xTb = np.ascontiguousarray(x[b].T)
        sel = np.flatnonzero(mask[b])[:KP]
        ku = len(sel)
        xkTb = np.zeros((HID, KP), np.float32)
        xkTb[:, :ku] = xTb[:, sel]
        mb = np.zeros(KP, np.float32)
        mb[ku:] = -1e9
        per_batch.append((xTb.astype(ml_dtypes.bfloat16),
                          xkTb.astype(ml_dtypes.bfloat16), mb))
    per_g = []
    for g in range(2):
        sl = slice(g * JC, (g + 1) * JC)
        cpack = np.zeros((128, CP_W), np.float32)
        cpack[:, CP_BVR:CP_BVR + 512] = np.asarray(bv)[sl].astype(np.float32)[None, :]
        cpack[:, CP_BQ:CP_BQ + 4] = np.asarray(bq)[sl].reshape(4, 128).T
        cpack[:, CP_BK:CP_BK + 4] = np.asarray(bk)[sl].reshape(4, 128).T
        bog = np.asarray(bo) if g == 0 else np.zeros(HID, np.float32)
        cpack[:, CP_BO:CP_BO + 8] = np.asarray(bog, np.float32).reshape(8, 128).T
        per_g.append({
            "wqT": np.ascontiguousarray(np.asarray(Wq)[sl].T.astype(ml_dtypes.bfloat16)),
            "wkT": np.ascontiguousarray(np.asarray(Wk)[sl].T.astype(ml_dtypes.bfloat16)),
            "wvT": np.ascontiguousarray(np.asarray(Wv)[sl].T.astype(ml_dtypes.bfloat16)),
            "woT": np.ascontiguousarray(
                np.asarray(Wo)[:, sl].T.astype(ml_dtypes.bfloat16)
            ),
            "cpack": cpack,
        })
    in_maps = []
    for c in range(NCORES):
        b, g = c // 2, c % 2
        xTb, xkTb, mb = per_batch[b]
        m = {"xT": xTb, "xkT": xkTb}
        m.update(per_g[g])
        cpk = per_g[g]["cpack"].copy()
        cpk[:, CP_MASK:CP_MASK + NKT] = mb.reshape(NKT, 128).T
        m["cpack"] = cpk
        in_maps.append(m)
    return in_maps


def kernel(x, mask, Wq, bq, Wk, bk, Wv, bv, Wo, bo):
    from concourse.bass_utils import run_bass_kernel_spmd

    nc = _get_nc()
    in_maps = make_in_maps(x, mask, Wq, bq, Wk, bk, Wv, bv, Wo, bo)
    kw = {}
    if TRACE:
        import shutil
        shutil.rmtree("/root/problem/trace_out", ignore_errors=True)
        os.makedirs("/root/problem/trace_out", exist_ok=True)
        kw = dict(tmpdir="/root/problem/trace_out")
    r = run_bass_kernel_spmd(nc, in_maps, list(range(NCORES)), trace=TRACE, **kw)
    LAST_RESULTS["exec_time_ns"] = r.exec_time_ns
    LAST_RESULTS["mean_exec_time_ns"] = r.mean_exec_time_ns
    y = np.empty((B, S, HID), np.float32)
    for b in range(B):
        y[b] = (np.asarray(r.results[2 * b]["y"], np.float32)
                + np.asarray(r.results[2 * b + 1]["y"], np.float32)).T
    return y


# revision 21
# speedup vs baseline: 1.1669x; 1.0054x over previous
"""Trainium2 Bass kernel for nn_AttentionModeEncoder (B=4, S=2048, HID=1024, 16 heads x 64).

Sharding: 8 cores = 4 batches x 2 head-groups (8 heads / 512 features per core).

Design notes (R1 rev):
  - All transposes on HOST: xT, compacted xkT, WqT/WkT/WvT, WoT are DRAM
    inputs (bf16).  Mask compaction on host: only unmasked keys (~1044 of
    2048) shipped, padded to KP=1152; pad slots get -1e9 bias.
  - Everything bf16 on the PE; PSUM accumulation fp32; softmax denominator
    fp32 via the [V | ones] augmented AV stationary matrix.
  - Phase B is a 3-engine software pipeline: scores for iteration i+1 are
    emitted one full iteration ahead so the ACT engine's exp stream never
    stalls; AV matmuls trail each exp per key-tile; DVE does the
    reciprocal-normalize per iteration.
  - DMA triggers are split across the two HWDGE queues (sync + scalar):
    sync carries the A1-critical xk/wv/wk and half the y writes; scalar
    carries the consts pack, wq, wo, gated xq slices, and the other half
    of y.  y is written bf16 (host sums the two partials per batch).
Per core (batch b, head-group g):
  A1: K^T/V projections from xkT, V in [k, head, d|ones] layout.
  A2: Q^T projection from xT (jt-outer), bias on DVE.
  B:  pipelined scores/exp/AV/normalize per (head, 1024-wide q chunk).
  C:  partial out-projection y^T = WoT^T @ attn^T, bias, bf16 DMA out.
"""


import os
import sys
import numpy as np
from contextlib import ExitStack

for _p in ("/opt/trn_rl_repo", "/root/.axon_site/_ro/trn_rl_repo"):
    if os.path.isdir(_p) and _p not in sys.path:
        sys.path.insert(0, _p)

import ml_dtypes
import concourse.bass as bass
import concourse.bacc as bacc
import concourse.mybir as mybir
import concourse.tile as tile
from concourse import library_config

B, S, HID = 4, 2048, 1024
JC = 512                 # features per core (8 heads)
KP = 1152                # compacted+padded key slots (9 k-tiles)
NKT = KP // 128          # 9
NCORES = 8
FP = mybir.dt.float32
FR = mybir.dt.float32r
BF = mybir.dt.bfloat16
F8 = mybir.dt.float8e4
DR = mybir.MatmulPerfMode.DoubleRow
MULT = mybir.AluOpType.mult
ADD = mybir.AluOpType.add
EXP = mybir.ActivationFunctionType.Exp
IDENT = mybir.ActivationFunctionType.Identity

# consts pack layout (fp32 columns)
CP_BVR = 0      # [128, 512] bv replicated
CP_MASK = 512   # [128, 9]   mask bias per key tile
CP_BQ = 521     # [128, 4]   bq transposed
CP_BK = 525     # [128, 4]   bk transposed
CP_BO = 529     # [128, 8]   bo transposed
CP_W = 537

TRACE = False
LAST_RESULTS = {}


def build_nc():
    nc = bacc.Bacc()
    xT = nc.declare_dram_parameter("xT", [HID, S], BF, isOutput=False)
    xkT = nc.declare_dram_parameter("xkT", [HID, KP], BF, isOutput=False)
    cpack = nc.declare_dram_parameter("cpack", [128, CP_W], FP, isOutput=False)
    wqT = nc.declare_dram_parameter("wqT", [HID, JC], BF, isOutput=False)
    wkT = nc.declare_dram_parameter("wkT", [HID, JC], BF, isOutput=False)
    wvT = nc.declare_dram_parameter("wvT", [HID, JC], BF, isOutput=False)
    woT = nc.declare_dram_parameter("woT", [JC, HID], BF, isOutput=False)
    y = nc.declare_dram_parameter("y", [HID, S], BF, isOutput=True)

    with tile.TileContext(nc) as tc, ExitStack() as ctx:
        const = ctx.enter_context(tc.tile_pool(name="const", bufs=1))
        mid = ctx.enter_context(tc.tile_pool(name="mid", bufs=1))
        wop = ctx.enter_context(tc.tile_pool(name="wop", bufs=1))
        # LIFO pool stacks: a1 (innermost) closes after A1, a12 after A2
        a12stack = ExitStack()
        xqp = a12stack.enter_context(tc.tile_pool(name="xqp", bufs=1))
        wqp = a12stack.enter_context(tc.tile_pool(name="wqp", bufs=1))
        a1stack = ExitStack()
        xkp = a1stack.enter_context(tc.tile_pool(name="xkp", bufs=1))
        wkvp = a1stack.enter_context(tc.tile_pool(name="wkvp", bufs=1))

        # persistent activations
        QTs = mid.tile([128, 4, S], BF)          # [j-in-tile, jt, t]   16KB/part
        KTs = mid.tile([128, 4, KP], BF)         # [j-in-tile, jt, kc]   9KB
        vaug = mid.tile([128, NKT, 8, 128], BF)  # [kc, kt, head, d|ones@64|pad]
        nc.gpsimd.memset(vaug[:, :, :, 64:65], 1.0)
        # head 7 (the last two B iterations) gets a full 64-wide ones block so
        # its finalize skips the gpsimd broadcast on the critical tail
        nc.gpsimd.memset(vaug[:, :, 7, 64:128], 1.0)
        outT = mid.tile([128, 4, S], BF)         # attn out^T [c, ct, t] 16KB

        # --- A1-critical loads split across both HWDGE queues so the first
        # v_proj inputs (xk slice 0 + wv) transfer in parallel ---------------
        xk = xkp.tile([128, 8, KP], BF)          # 18KB, freed after A1
        xk_r = xkT.rearrange("(it p) k -> p it k", p=128)
        wv_sb = wkvp.tile([128, 8, JC], BF)
        wv_r = wvT.rearrange("(it p) j -> p it j", p=128)
        # first v_proj chain needs wv it-tiles in order and xk cols 0:128:
        # land those halves first, split across both HWDGE queues
        nc.sync.dma_start(out=wv_sb[:, 0:4, :], in_=wv_r[:, 0:4, :])
        nc.scalar.dma_start(out=xk[:, :, 0:256], in_=xk_r[:, :, 0:256])
        nc.sync.dma_start(out=wv_sb[:, 4:8, :], in_=wv_r[:, 4:8, :])
        wk_sb = wkvp.tile([128, 8, JC], BF)
        for k0, k1 in ((256, 512), (512, 832), (832, KP)):
            nc.sync.dma_start(out=xk[:, :, k0:k1], in_=xk_r[:, :, k0:k1])
        nc.sync.dma_start(out=wk_sb[:], in_=wkT.rearrange("(it p) j -> p it j", p=128))

        # --- everything else on the scalar HWDGE queue ---------------------
        cp = const.tile([128, CP_W], FP)
        nc.scalar.dma_start(out=cp[:], in_=cpack[:, :])
        wq_sb = wqp.tile([128, 8, JC], BF)
        nc.scalar.dma_start(out=wq_sb[:], in_=wqT.rearrange("(it p) j -> p it j", p=128))
        wo_sb = wop.tile([128, 4, HID], BF)
        nc.scalar.dma_start(out=wo_sb[:], in_=woT.rearrange("(ct p) o -> p ct o", p=128))
        xq = xqp.tile([128, 8, S], BF)           # 32KB, freed after A2

        bvr = cp[:, CP_BVR:CP_BVR + 512]
        maskA = cp[:, CP_MASK:CP_MASK + NKT]
        bqt = cp[:, CP_BQ:CP_BQ + 4]
        bkt = cp[:, CP_BK:CP_BK + 4]
        bot = cp[:, CP_BO:CP_BO + 8]

        # ---------------- Phase A1: K^T and V projections (compacted keys) ---
        with ExitStack() as actx:
            psA = actx.enter_context(tc.tile_pool(name="psA", bufs=2, space="PSUM"))

            # V in natural [kc, head, d] layout: out[kc, j] = sum_i xkT[i,kc] WvT[i,j]
            def v_proj(kc):
                ps = psA.tile([128, 8, 64], FP, tag="psv")
                for it in range(8):
                    nc.tensor.matmul(
                        ps[:],
                        lhsT=xk[:, it, kc * 128:(kc + 1) * 128],
                        rhs=wv_sb[:, it, :],
                        start=(it == 0), stop=(it == 7),
                    )
                nc.vector.tensor_tensor(vaug[:, kc, :, 0:64], ps[:], bvr, ADD)

            # stagger the 4MB xq load across A1: each xq slice's DMA is gated
            # (WAW via gpsimd copy) on successive V tiles so startup bandwidth
            # goes to the A1-critical xk/wv/wk bytes first.
            xT_r = xT.rearrange("(it p) t -> p it t", p=128)

            def xq_gate(tq, kc):
                t0 = tq * 512
                nc.gpsimd.tensor_copy(
                    out=xq[0:1, 0, t0:t0 + 1], in_=vaug[0:1, kc, 0, 0:1]
                )
                nc.scalar.dma_start(
                    out=xq[:, :, t0:t0 + 512], in_=xT_r[:, :, t0:t0 + 512]
                )

            v_proj(0)
            xq_gate(0, 0)
            for kc in range(1, NKT):
                v_proj(kc)
                if kc in (2, 4, 6):
                    xq_gate(kc // 2, kc)

            # K^T[j, kc]: 3 chunks of 384 columns
            for jt in range(4):
                for cc in range(3):
                    c0 = cc * 384
                    ps = psA.tile([128, 384], FP, tag="psk")
                    for it in range(8):
                        nc.tensor.matmul(
                            ps[:],
                            lhsT=wk_sb[:, it, jt * 128:(jt + 1) * 128],
                            rhs=xk[:, it, c0:c0 + 384],
                            start=(it == 0), stop=(it == 7),
                        )
                    nc.vector.tensor_scalar_add(
                        KTs[:, jt, c0:c0 + 384], ps[:], bkt[:, jt:jt + 1]
                    )

        a1stack.close()

        # ---------------- Phase A2: Q^T projection (jt-outer, bias on DVE) ---
        with ExitStack() as actx:
            psQ = actx.enter_context(tc.tile_pool(name="psQ", bufs=2, space="PSUM"))
            for jt in range(4):
                for tq in range(4):
                    t0 = tq * 512
                    ps = psQ.tile([128, 512], FP, tag="psq")
                    for it in range(8):
                        nc.tensor.matmul(
                            ps[:],
                            lhsT=wq_sb[:, it, jt * 128:(jt + 1) * 128],
                            rhs=xq[:, it, t0:t0 + 512],
                            start=(it == 0), stop=(it == 7),
                        )
                    nc.vector.tensor_scalar_add(
                        QTs[:, jt, t0:t0 + 512], ps[:], bqt[:, jt:jt + 1]
                    )

        a12stack.close()

        # gpsimd: switch from the standard ucode library to `attn` for the
        # partition_broadcast used in phase B (after the last std-lib copy)
        nc.gpsimd.load_library(library_config.attn)

        # ---------------- Phase B: attention (3-engine pipeline) -------------
        # One N=1024 matmul per key-tile for both scores and AV (2 PSUM
        # banks per write).  Per exp slot the PE does one half-array scores
        # matmul + one full-array AV matmul of the previous iteration, which
        # keeps instantaneous PE power smooth (the hw governor clamps the PE
        # to 50% util when bursty full-array chains push it over budget).
        with ExitStack() as bctx:
            ptp = bctx.enter_context(tc.tile_pool(name="ptp", bufs=2))
            rp = bctx.enter_context(tc.tile_pool(name="rp", bufs=2))
            spool = bctx.enter_context(tc.tile_pool(name="spool", bufs=2, space="PSUM"))
            avpool = bctx.enter_context(tc.tile_pool(name="avpool", bufs=2, space="PSUM"))

            iters = [(h, qc) for h in range(8) for qc in range(2)]
            sp_t, PT_t, avp_t = {}, {}, {}

            def scores(i, kt):
                h, qc = iters[i]
                jt, p0, q0 = h // 2, (h % 2) * 64, qc * 1024
                sp = spool.tile([128, 1024], FP, tag="sp", name="sp")
                sp_t[(i, kt)] = sp
                for qq in range(2):
                    nc.tensor.matmul(
                        sp[:, qq * 512:(qq + 1) * 512],
                        lhsT=KTs[p0:p0 + 64, jt, kt * 128:(kt + 1) * 128],
                        rhs=QTs[p0:p0 + 64, jt, q0 + qq * 512:q0 + (qq + 1) * 512],
                        start=True, stop=True,
                    )

            def expf(i, kt):
                if kt == 0:
                    PT_t[i] = ptp.tile([128, NKT, 1024], BF, tag="PT", name="PT")
                nc.scalar.activation(
                    PT_t[i][:, kt, :], sp_t.pop((i, kt))[:], EXP,
                    bias=maskA[:, kt:kt + 1], scale=0.125,
                )

            def av_chain(i, qq):
                h, _ = iters[i]
                wid = 128 if h == 7 else 65
                if qq == 0:
                    avp_t[i] = avpool.tile([128, 1024], FP, tag="avp", name="avp")
                for kt in range(NKT):
                    nc.tensor.matmul(
                        avp_t[i][0:wid, qq * 512:(qq + 1) * 512],
                        lhsT=vaug[:, kt, h, 0:wid],
                        rhs=PT_t[i][:, kt, qq * 512:(qq + 1) * 512],
                        start=(kt == 0), stop=(kt == NKT - 1),
                        skip_group_check=True,
                    )

            def dencopy(i):
                # denominator row(s) PSUM -> SBUF (custom-DVE recip must read
                # SBUF on real hw).  Head 7 has the full replicated block.
                h, _ = iters[i]
                rows = 64 if h == 7 else 1
                den1 = rp.tile([rows, 1024], FP, tag="den1", name="den1")
                den_t[i] = den1
                nc.vector.tensor_copy(out=den1[:], in_=avp_t[i][64:64 + rows, :])

            def recip(i):
                h, _ = iters[i]
                rows = 64 if h == 7 else 1
                rec1 = rp.tile([rows, 1024], FP, tag="rec1", name="rec1")
                rec_t[i] = rec1
                nc.vector.reciprocal_approx_fast(rec1[:], den_t.pop(i)[:])
                if h == 7:
                    rb_t[i] = rec_t.pop(i)

            def bcast(i):
                # broadcast rec1 [1,1024] to 64 partitions on the idle gpsimd
                if i in rb_t:
                    return        # head-7 fast path: already 64 rows
                rec1 = rec_t.pop(i)
                rb = rp.tile([64, 1024], FP, tag="rb", name="rb")
                rb_t[i] = rb
                nc.gpsimd.partition_broadcast(rb[:], rec1[:], channels=64)

            def normalize(i):
                h, qc = iters[i]
                jt, p0, q0 = h // 2, (h % 2) * 64, qc * 1024
                avp = avp_t.pop(i)
                nc.vector.tensor_tensor(
                    outT[p0:p0 + 64, jt, q0:q0 + 1024],
                    avp[0:64, :], rb_t.pop(i)[:], MULT,
                )
                PT_t.pop(i)

            rec_t, rb_t, den_t = {}, {}, {}
            for i in range(17):
                for kt in range(NKT):
                    if i < 16:
                        scores(i, kt)
                        expf(i, kt)
                    if i >= 1:
                        if kt == 1:
                            av_chain(i - 1, 0)
                        elif kt == 5:
                            av_chain(i - 1, 1)
                        elif kt == 6:
                            dencopy(i - 1)
                            recip(i - 1)
                        elif kt == 7:
                            bcast(i - 1)
                        elif kt == 8:
                            normalize(i - 1)

        # ---------------- Phase C: partial out-projection --------------------
        with ExitStack() as cctx:
            ypool = cctx.enter_context(tc.tile_pool(name="ypool", bufs=4))
            ypsum = cctx.enter_context(tc.tile_pool(name="ypsum", bufs=3, space="PSUM"))

            for ot in range(8):
                for tch in range(2):
                    t0 = tch * 1024
                    yps = ypsum.tile([128, 1024], FP, tag="yps")
                    for qq in range(2):
                        for ct in range(4):
                            nc.tensor.matmul(
                                yps[:, qq * 512:(qq + 1) * 512],
                                lhsT=wo_sb[:, ct, ot * 128:(ot + 1) * 128],
                                rhs=outT[:, ct, t0 + qq * 512:t0 + (qq + 1) * 512],
                                start=(ct == 0), stop=(ct == 3),
                            )
                    yt = ypool.tile([128, 1024], BF, tag="yt")
                    if (ot + tch) % 2 == 0:
                        nc.scalar.activation(
                            yt[:], yps[:], IDENT, bias=bot[:, ot:ot + 1], scale=1.0
                        )
                    else:
                        nc.vector.tensor_scalar_add(yt[:], yps[:], bot[:, ot:ot + 1])
                    eng = nc.sync if (ot + tch) % 2 == 0 else nc.scalar
                    eng.dma_start(
                        out=y[ot * 128:(ot + 1) * 128, t0:t0 + 1024], in_=yt[:],
                    )
    return nc


_NC = None


def _get_nc():
    global _NC
    if _NC is None:
        _NC = build_nc()
        _NC.finalize()   # run Bacc passes (reg alloc, wait splitting)
    return _NC


def make_in_maps(x, mask, Wq, bq, Wk, bk, Wv, bv, Wo, bo):
    x = np.asarray(x, np.float32)
    mask = np.asarray(mask)
    per_batch = []
    for b in range(B):
        xTb = np.ascontiguousarray(x[b].T)
        sel = np.flatnonzero(mask[b])[:KP]
        ku = len(sel)
        xkTb = np.zeros((HID, KP), np.float32)
        xkTb[:, :ku] = xTb[:, sel]
        mb = np.zeros(KP, np.float32)
        mb[ku:] = -1e9
        per_batch.append((xTb.astype(ml_dtypes.bfloat16),
                          xkTb.astype(ml_dtypes.bfloat16), mb))
    per_g = []
    for g in range(2):
        sl = slice(g * JC, (g + 1) * JC)
        cpack = np.zeros((128, CP_W), np.float32)
        cpack[:, CP_BVR:CP_BVR + 512] = np.asarray(bv)[sl].astype(np.float32)[None, :]
        cpack[:, CP_BQ:CP_BQ + 4] = np.asarray(bq)[sl].reshape(4, 128).T
        cpack[:, CP_BK:CP_BK + 4] = np.asarray(bk)[sl].reshape(4, 128).T
        bog = np.asarray(bo) if g == 0 else np.zeros(HID, np.float32)
        cpack[:, CP_BO:CP_BO + 8] = np.asarray(bog, np.float32).reshape(8, 128).T
        per_g.append({
            "wqT": np.ascontiguousarray(np.asarray(Wq)[sl].T.astype(ml_dtypes.bfloat16)),
            "wkT": np.ascontiguousarray(np.asarray(Wk)[sl].T.astype(ml_dtypes.bfloat16)),
            "wvT": np.ascontiguousarray(np.asarray(Wv)[sl].T.astype(ml_dtypes.bfloat16)),
            "woT": np.ascontiguousarray(
                np.asarray(Wo)[:, sl].T.astype(ml_dtypes.bfloat16)
            ),
            "cpack": cpack,
        })
    in_maps = []
    for c in range(NCORES):
        b, g = c // 2, c % 2
        xTb, xkTb, mb = per_batch[b]
        m = {"xT": xTb, "xkT": xkTb}
        m.update(per_g[g])
        cpk = per_g[g]["cpack"].copy()
        cpk[:, CP_MASK:CP_MASK + NKT] = mb.reshape(NKT, 128).T
        m["cpack"] = cpk
        in_maps.append(m)
    return in_maps


def kernel(x, mask, Wq, bq, Wk, bk, Wv, bv, Wo, bo):
    from concourse.bass_utils import run_bass_kernel_spmd

    nc = _get_nc()
    in_maps = make_in_maps(x, mask, Wq, bq, Wk, bk, Wv, bv, Wo, bo)
    kw = {}
    if TRACE:
        import shutil
        shutil.rmtree("/root/problem/trace_out", ignore_errors=True)
        os.makedirs("/root/problem/trace_out", exist_ok=True)
        kw = dict(tmpdir="/root/problem/trace_out")
    r = run_bass_kernel_spmd(nc, in_maps, list(range(NCORES)), trace=TRACE, **kw)
    LAST_RESULTS["exec_time_ns"] = r.exec_time_ns
    LAST_RESULTS["mean_exec_time_ns"] = r.mean_exec_time_ns
    y = np.empty((B, S, HID), np.float32)
    for b in range(B):
        y[b] = (np.asarray(r.results[2 * b]["y"], np.float32)
                + np.asarray(r.results[2 * b + 1]["y"], np.float32)).T
    return y
